# revision 18
# baseline (speedup 1.0000x reference)
"""DeltaSynapse kernel for Trainium2 (8 NeuronCores, SPMD).

Reference computation:
    Xpre[b,e,o] = sum_d delaymap[d,e,o] * Xd[d,b,e]
    I[b,o]      = sum_e (signs*W)[e,o] * Xpre[b,e,o]

Folded:  I[b,o] = sum_{d,e} (delaymap[d,e,o] * Weff[e,o]) * Xd[d,b,e]
i.e. a sum of D matmuls  I += Xd[d] @ (delaymap[d] . Weff).

Sharding: shard the contraction (pre-neuron e) dim across the 8 cores
(256 rows each). Each core reads its own e-slice of delaymap/W/signs/Xd
and produces a full [16, 2048] partial output; the host sums the 8
partials.

Implementation notes (hardware-measured tuning):
- Host shards are fp16 (delaymap one-hot 0/1 is exact in fp16; the
  device computed in fp16 from the start, so this halves HBM traffic
  without changing numerics). All input DMAs are plain HWDGE on the
  sync queue; W/signs land first so Weff=W*signs (DVE) finishes while
  delaymap slab 0 is still in flight.
- One DMA per (slab, e-chunk): each completion semaphore then covers
  half the descriptors, so a single straggler SDMA engine (of the 16
  that must all increment the sem) barely delays the consumer
  multiply. This collapsed multi-microsecond stalls and run-to-run
  variance. More, smaller DMAs regress: >~14 in-flight DMAs exhaust
  the 8 DMA-sem lanes and serialize later DMA issues.
- The dm*weff multiplies read flat [P, D, w] regions (slicing the
  d-dim costs ~40% DVE throughput) and run 2x mode at ~0.56 ns/elem;
  the DVE chain is DMA-paced end to end.
- PE pre-warmed with dummy matmuls so the HAM clock gate (1.2 vs
  2.4 GHz) is open when the real matmul stream starts; 12 dummies
  bridge the idle gap (the gate re-closes after ~3.4 us idle).
- o-ranges taper (512x3, 256, 128x2) so the post-stream tail
  (last multiply + 16 matmuls + copy + output DMA) stays short.
"""

import numpy as np

D, B, N = 8, 16, 2048
NCORES = 8
P = 128                 # SBUF partitions / matmul contraction tile
ESH = N // NCORES       # per-core pre-dim shard = 256
ECH = ESH // P          # e-chunks per core = 2

# DMA slabs: (name, [list of o-ranges]); each slab tile is
# [P, NR, ECH, D, w] with equal-width ranges stacked flat.
DMA_SLABS = [
    ("dm0", [(0, 512)]),
    ("dm1", [(512, 1024)]),
    ("dm2", [(1024, 1536)]),
    ("dm3", [(1536, 1792)]),
    ("dm4", [(1792, 1920), (1920, 2048)]),
]
NWARM = 12              # dummy matmuls to open the PE clock gate


def _build_program():
    from concourse import bacc, tile
    from concourse import mybir

    f32 = mybir.dt.float32
    f16 = mybir.dt.float16

    nc = bacc.Bacc()
    # Host-prepared layouts (see kernel() below), all fp16 in HBM:
    #   dm{i}: [P, NR, ECH, D, w]  delaymap o-slab (flat per range/chunk)
    #   wsa/wsb: [P, 2, N]         (W, signs) rows, e-chunk a/b
    #   xd   : [P, ECH, D, B]      Xd slice transposed
    dram = {}
    for name, ranges in DMA_SLABS:
        w = ranges[0][1] - ranges[0][0]
        dram[name] = nc.dram_tensor(
            name, [P, len(ranges), ECH, D, w], f16, kind="ExternalInput"
        )
    wsa = nc.dram_tensor("wsa", [P, 2, N], f16, kind="ExternalInput")
    wsb = nc.dram_tensor("wsb", [P, 2, N], f16, kind="ExternalInput")
    xd = nc.dram_tensor("xd", [P, ECH, D, B], f16, kind="ExternalInput")
    out = nc.dram_tensor("out", [B, N], f32, kind="ExternalOutput")

    with tile.TileContext(nc) as tc:
        with (
            tc.tile_pool(name="const", bufs=1) as cpool,
            tc.tile_pool(name="dm", bufs=5) as dmpool,
            tc.tile_pool(name="wd", bufs=6) as wdpool,
            tc.tile_pool(name="psum", bufs=1, space="PSUM") as ppool,
            tc.tile_pool(name="outp", bufs=6) as opool,
        ):
            wsa_t = cpool.tile([P, 2, N], f16)
            wsb_t = cpool.tile([P, 2, N], f16)
            weff = cpool.tile([P, ECH, N], f16)
            xd_t = cpool.tile([P, ECH, D, B], f16)

            dm_tiles = {}
            for name, ranges in DMA_SLABS:
                w = ranges[0][1] - ranges[0][0]
                dm_tiles[name] = dmpool.tile(
                    [P, len(ranges), ECH, D, w], f16, tag="dmslab", name=name
                )

            # HWDGE input DMAs on the sync queue. Wire order: xd/wsa
            # first (gate prewarm + weff_a), first dm chunk next, wsb
            # hidden behind it, then the o-major chunk stream. One DMA
            # per (slab, chunk) [tail slab: per (range, chunk)] keeps
            # completion-sem descriptor sets small.
            nc.sync.dma_start(wsa_t[:], wsa[:])
            nc.sync.dma_start(wsb_t[:], wsb[:])
            nc.sync.dma_start(xd_t[:], xd[:])
            for name, ranges in DMA_SLABS:
                for c in range(ECH):
                    nc.sync.dma_start(dm_tiles[name][:, :, c],
                                      dram[name][:, :, c])

            nc.vector.tensor_mul(weff[:, 0, :], wsa_t[:, 0], wsa_t[:, 1])
            nc.vector.tensor_mul(weff[:, 1, :], wsb_t[:, 0], wsb_t[:, 1])

            # PE pre-warm: dummy matmuls on already-landed tiles open the
            # HAM clock gate (2.4 GHz) before the real matmul stream.
            pwarm = ppool.tile([B, 512], f32, tag="pwarm")
            for _ in range(NWARM):
                nc.tensor.matmul(
                    pwarm[:], xd_t[:, 0, 0, :], wsa_t[:, 0, 0:512],
                    start=True, stop=True,
                )

            psum = ppool.tile([B, N], f32)
            for name, ranges in DMA_SLABS:
                dm_t = dm_tiles[name]
                for r, (o0, o1) in enumerate(ranges):
                    olen = o1 - o0
                    wd_ts = []
                    for c in range(ECH):
                        wd_t = wdpool.tile([P, D, olen], f16, tag="wd")
                        nc.vector.tensor_mul(
                            wd_t[:],
                            dm_t[:, r, c],
                            weff[:, c, o0:o1].unsqueeze(1).broadcast_to(
                                [P, D, olen]
                            ),
                        )
                        wd_ts.append(wd_t)
                    for c in range(ECH):
                        for d in range(D):
                            nc.tensor.matmul(
                                psum[:, o0:o1],
                                xd_t[:, c, d, :],
                                wd_ts[c][:, d, :],
                                start=(c == 0 and d == 0),
                                stop=(c == ECH - 1 and d == D - 1),
                            )
                    out_t = opool.tile([B, olen], f32, tag="out",
                                       name=f"o{o0}")
                    nc.scalar.copy(out_t[:], psum[:, o0:o1])
                    nc.scalar.dma_start(out[:, o0:o1], out_t[:])

    nc.compile()
    return nc


_prog_cache = {}


def _get_program():
    if "nc" not in _prog_cache:
        _prog_cache["nc"] = _build_program()
    return _prog_cache["nc"]


def _shard_inputs(Xd, delaymap, W, signs):
    """Layout permutation/slicing + fp16 cast -> per-core input maps."""
    Xd = np.asarray(Xd, dtype=np.float32)
    delaymap = np.asarray(delaymap, dtype=np.float32)
    W = np.asarray(W, dtype=np.float32)
    signs = np.asarray(signs, dtype=np.float32)

    in_maps = []
    for k in range(NCORES):
        esl = slice(k * ESH, (k + 1) * ESH)
        # delaymap [D, ESH, N] -> [P, ECH, D, N] fp16
        dm_pcd = (
            delaymap[:, esl, :]
            .reshape(D, ECH, P, N)
            .transpose(2, 1, 0, 3)
            .astype(np.float16)
        )
        m = {}
        for name, ranges in DMA_SLABS:
            # [P, NR, ECH, D, w]
            m[name] = np.ascontiguousarray(
                np.stack([dm_pcd[:, :, :, o0:o1] for o0, o1 in ranges],
                         axis=1)
            )
        # W/signs rows per e-chunk -> [P, 2, N] fp16 each
        wk = W[esl].reshape(ECH, P, N).astype(np.float16)
        sk = signs[esl].reshape(ECH, P, N).astype(np.float16)
        m["wsa"] = np.ascontiguousarray(np.stack([wk[0], sk[0]], axis=1))
        m["wsb"] = np.ascontiguousarray(np.stack([wk[1], sk[1]], axis=1))
        # Xd [D, B, ESH] -> [P, ECH, D, B] fp16
        m["xd"] = np.ascontiguousarray(
            Xd[:, :, esl].reshape(D, B, ECH, P).transpose(3, 2, 0, 1)
        ).astype(np.float16)
        in_maps.append(m)
    return in_maps


def _run(in_maps, trace=False, **kw):
    from concourse.bass_utils import run_bass_kernel_spmd

    nc = _get_program()
    return run_bass_kernel_spmd(nc, in_maps, list(range(NCORES)), trace=trace, **kw)


def _gather(res):
    acc = np.zeros((B, N), dtype=np.float64)
    for k in range(NCORES):
        acc += res.results[k]["out"].astype(np.float64)
    return acc.astype(np.float32)


def kernel(Xd, X, delaymap, W, signs):
    in_maps = _shard_inputs(Xd, delaymap, W, signs)
    return _gather(_run(in_maps))


# revision 20
# speedup vs baseline: 1.0879x; 1.0879x over previous
"""DeltaSynapse kernel for Trainium2 (8 NeuronCores, SPMD).

Reference computation:
    Xpre[b,e,o] = sum_d delaymap[d,e,o] * Xd[d,b,e]
    I[b,o]      = sum_e (signs*W)[e,o] * Xpre[b,e,o]

Folded:  I[b,o] = sum_{d,e} (delaymap[d,e,o] * Weff[e,o]) * Xd[d,b,e]
i.e. a sum of D matmuls  I += Xd[d] @ (delaymap[d] . Weff).

Sharding: shard the contraction (pre-neuron e) dim across the 8 cores
(256 rows each). Each core reads its own e-slice of delaymap/W/signs/Xd
and produces a full [16, 2048] partial output; the host sums the 8
partials.

Implementation notes (hardware-measured tuning):
- Host shards are fp16 (delaymap one-hot 0/1 is exact in fp16; the
  device computed in fp16 from the start, so this halves HBM traffic
  without changing numerics). All input DMAs are plain HWDGE on the
  sync queue; W/signs land first so Weff=W*signs (DVE) finishes while
  delaymap slab 0 is still in flight.
- One DMA per (slab, e-chunk): each completion semaphore then covers
  half the descriptors, so a single straggler SDMA engine (of the 16
  that must all increment the sem) barely delays the consumer
  multiply. This collapsed multi-microsecond stalls and run-to-run
  variance. More, smaller DMAs regress: >~14 in-flight DMAs exhaust
  the 8 DMA-sem lanes and serialize later DMA issues.
- The dm*weff multiplies read flat [P, D, w] regions (slicing the
  d-dim costs ~40% DVE throughput) and run 2x mode at ~0.56 ns/elem;
  the DVE chain is DMA-paced end to end.
- PE pre-warmed with dummy matmuls so the HAM clock gate (1.2 vs
  2.4 GHz) is open when the real matmul stream starts; 12 dummies
  bridge the idle gap (the gate re-closes after ~3.4 us idle).
- o-ranges taper (512x3, 256, 128x2) so the post-stream tail
  (last multiply + 16 matmuls + copy + output DMA) stays short.
"""

import numpy as np

D, B, N = 8, 16, 2048
NCORES = 8
P = 128                 # SBUF partitions / matmul contraction tile
ESH = N // NCORES       # per-core pre-dim shard = 256
ECH = ESH // P          # e-chunks per core = 2

# DMA slabs: (name, [list of o-ranges]); each slab tile is
# [P, NR, ECH, D, w] with equal-width ranges stacked flat.
DMA_SLABS = [
    ("dm0", [(0, 512)]),
    ("dm1", [(512, 1024)]),
    ("dm2", [(1024, 1536)]),
    ("dm3", [(1536, 1792)]),
    ("dm4", [(1792, 1920), (1920, 2048)]),
]
LAST = (1920, 2048)     # dual-PSUM range
NWARM = 12              # dummy matmuls to open the PE clock gate


def _build_program():
    from concourse import bacc, tile
    from concourse import mybir

    f32 = mybir.dt.float32
    f16 = mybir.dt.float16

    nc = bacc.Bacc()
    # Host-prepared layouts (see kernel() below), all fp16 in HBM:
    #   dm{i}: [P, NR, ECH, D, w]  delaymap o-slab (flat per range/chunk)
    #   wsa/wsb: [P, 2, N]         (W, signs) rows, e-chunk a/b
    #   xd   : [P, ECH, D, B]      Xd slice transposed
    dram = {}
    for name, ranges in DMA_SLABS:
        w = ranges[0][1] - ranges[0][0]
        dram[name] = nc.dram_tensor(
            name, [P, len(ranges), ECH, D, w], f16, kind="ExternalInput"
        )
    wsa = nc.dram_tensor("wsa", [P, 2, N], f16, kind="ExternalInput")
    wsb = nc.dram_tensor("wsb", [P, 2, N], f16, kind="ExternalInput")
    xd = nc.dram_tensor("xd", [P, ECH, D, B], f16, kind="ExternalInput")
    out = nc.dram_tensor("out", [B, N], f32, kind="ExternalOutput")

    with tile.TileContext(nc) as tc:
        with (
            tc.tile_pool(name="const", bufs=1) as cpool,
            tc.tile_pool(name="dm", bufs=5) as dmpool,
            tc.tile_pool(name="wd", bufs=6) as wdpool,
            tc.tile_pool(name="psum", bufs=1, space="PSUM") as ppool,
            tc.tile_pool(name="outp", bufs=6) as opool,
        ):
            wsa_t = cpool.tile([P, 2, N], f16)
            wsb_t = cpool.tile([P, 2, N], f16)
            weff = cpool.tile([P, ECH, N], f16)
            xd_t = cpool.tile([P, ECH, D, B], f16)

            dm_tiles = {}
            for name, ranges in DMA_SLABS:
                w = ranges[0][1] - ranges[0][0]
                dm_tiles[name] = dmpool.tile(
                    [P, len(ranges), ECH, D, w], f16, tag="dmslab", name=name
                )

            # HWDGE input DMAs on the sync queue. Wire order: xd/wsa
            # first (gate prewarm + weff_a), first dm chunk next, wsb
            # hidden behind it, then the o-major chunk stream. One DMA
            # per (slab, chunk) [tail slab: per (range, chunk)] keeps
            # completion-sem descriptor sets small.
            nc.sync.dma_start(wsa_t[:], wsa[:])
            nc.sync.dma_start(wsb_t[:], wsb[:])
            nc.sync.dma_start(xd_t[:], xd[:])
            for name, ranges in DMA_SLABS:
                for c in range(ECH):
                    nc.sync.dma_start(dm_tiles[name][:, :, c],
                                      dram[name][:, :, c])

            nc.vector.tensor_mul(weff[:, 0, :], wsa_t[:, 0], wsa_t[:, 1])
            nc.vector.tensor_mul(weff[:, 1, :], wsb_t[:, 0], wsb_t[:, 1])

            # PE pre-warm: dummy matmuls on already-landed tiles open the
            # HAM clock gate (2.4 GHz) before the real matmul stream.
            pwarm = ppool.tile([B, 512], f32, tag="pwarm")
            for _ in range(NWARM):
                nc.tensor.matmul(
                    pwarm[:], xd_t[:, 0, 0, :], wsa_t[:, 0, 0:512],
                    start=True, stop=True,
                )

            psum = ppool.tile([B, N], f32)
            psum2 = ppool.tile([B, LAST[1] - LAST[0]], f32)
            for name, ranges in DMA_SLABS:
                dm_t = dm_tiles[name]
                for r, (o0, o1) in enumerate(ranges):
                    olen = o1 - o0
                    last = (o0, o1) == LAST
                    wd_ts = []
                    for c in range(ECH):
                        wd_t = wdpool.tile([P, D, olen], f16, tag="wd")
                        nc.vector.tensor_mul(
                            wd_t[:],
                            dm_t[:, r, c],
                            weff[:, c, o0:o1].unsqueeze(1).broadcast_to(
                                [P, D, olen]
                            ),
                        )
                        wd_ts.append(wd_t)
                    if last:
                        # two independent accumulation groups (separate
                        # PSUM banks) for the final range's matmuls
                        for d in range(D):
                            for c, pt in ((0, psum[:, o0:o1]), (1, psum2[:])):
                                nc.tensor.matmul(
                                    pt, xd_t[:, c, d, :], wd_ts[c][:, d, :],
                                    start=(d == 0), stop=(d == D - 1),
                                )
                    else:
                        for c in range(ECH):
                            for d in range(D):
                                nc.tensor.matmul(
                                    psum[:, o0:o1],
                                    xd_t[:, c, d, :],
                                    wd_ts[c][:, d, :],
                                    start=(c == 0 and d == 0),
                                    stop=(c == ECH - 1 and d == D - 1),
                                )
                    out_t = opool.tile([B, olen], f32, tag="out",
                                       name=f"o{o0}")
                    if last:
                        # DVE may read only one PSUM operand per instr
                        nc.scalar.copy(out_t[:], psum[:, o0:o1])
                        nc.vector.tensor_add(out_t[:], out_t[:], psum2[:])
                    else:
                        nc.scalar.copy(out_t[:], psum[:, o0:o1])
                    nc.scalar.dma_start(out[:, o0:o1], out_t[:])

    nc.compile()
    return nc


_prog_cache = {}


def _get_program():
    if "nc" not in _prog_cache:
        _prog_cache["nc"] = _build_program()
    return _prog_cache["nc"]


def _shard_inputs(Xd, delaymap, W, signs):
    """Layout permutation/slicing + fp16 cast -> per-core input maps."""
    Xd = np.asarray(Xd, dtype=np.float32)
    delaymap = np.asarray(delaymap, dtype=np.float32)
    W = np.asarray(W, dtype=np.float32)
    signs = np.asarray(signs, dtype=np.float32)

    in_maps = []
    for k in range(NCORES):
        esl = slice(k * ESH, (k + 1) * ESH)
        # delaymap [D, ESH, N] -> [P, ECH, D, N] fp16
        dm_pcd = (
            delaymap[:, esl, :]
            .reshape(D, ECH, P, N)
            .transpose(2, 1, 0, 3)
            .astype(np.float16)
        )
        m = {}
        for name, ranges in DMA_SLABS:
            # [P, NR, ECH, D, w]
            m[name] = np.ascontiguousarray(
                np.stack([dm_pcd[:, :, :, o0:o1] for o0, o1 in ranges],
                         axis=1)
            )
        # W/signs rows per e-chunk -> [P, 2, N] fp16 each
        wk = W[esl].reshape(ECH, P, N).astype(np.float16)
        sk = signs[esl].reshape(ECH, P, N).astype(np.float16)
        m["wsa"] = np.ascontiguousarray(np.stack([wk[0], sk[0]], axis=1))
        m["wsb"] = np.ascontiguousarray(np.stack([wk[1], sk[1]], axis=1))
        # Xd [D, B, ESH] -> [P, ECH, D, B] fp16
        m["xd"] = np.ascontiguousarray(
            Xd[:, :, esl].reshape(D, B, ECH, P).transpose(3, 2, 0, 1)
        ).astype(np.float16)
        in_maps.append(m)
    return in_maps


def _run(in_maps, trace=False, **kw):
    from concourse.bass_utils import run_bass_kernel_spmd

    nc = _get_program()
    return run_bass_kernel_spmd(nc, in_maps, list(range(NCORES)), trace=trace, **kw)


def _gather(res):
    acc = np.zeros((B, N), dtype=np.float64)
    for k in range(NCORES):
        acc += res.results[k]["out"].astype(np.float64)
    return acc.astype(np.float32)


def kernel(Xd, X, delaymap, W, signs):
    in_maps = _shard_inputs(Xd, delaymap, W, signs)
    return _gather(_run(in_maps))
